# Initial kernel scaffold
#
"""Trainium2 Bass kernel for nn_AttentionModule (GNN message passing).

kernel(**inputs) takes the FULL unsharded inputs (as produced by
setup_inputs) and returns the FULL [B, 128] float32 output.

Strategy: data-parallel over graphs across 8 NeuronCores (batch is sorted, so
each core owns a contiguous range of graphs/nodes).  Per core, graphs are
packed into blocks of 32x128-node tiles with <= 18 graph slots; all segment
reductions are local matmuls against host-built one-hot slabs.

Math notes (big tensors bf16, accumulations fp32):
  att = tanh(z), z = relu(x@fc1.T+b1)@fc2.T (+b2==0)
  x2  = (1+att)*x = 2*sigma(2z)*x =: 2*y2       (sigmoid trick)
  meanT = (y2.T @ S) * (2/cnt);  tGT = tanh(Wm.T @ meanT)
  dots = y2 @ tGT;  coefs = sigma(2*dots);  outT = 2 * (y2.T @ (S*coefs))
"""

import sys
import numpy as np

sys.path.insert(0, "/opt/trn_rl_repo")

import ml_dtypes
from contextlib import ExitStack

import concourse.bass as bass
import concourse.bacc as bacc
import concourse.tile as tile
from concourse import mybir
from concourse.bass_utils import run_bass_kernel_spmd

BF = mybir.dt.bfloat16
# engine-balance knobs (out of 8): how many y2-multiplies go to GPSIMD,
# how many psum->sbuf copies go to ACT
Y2_GP_OF8 = 0
CP_ACT_OF8 = 0
PZ_WIDE = True        # one [128,1024] pz tile vs two-buffer [128,512]
RELU_ACT = True      # relu on ACT instead of DVE
XN_SCALAR_DMA = False # issue xn loads on the second HWDGE ring (ACT seq)
DMA_CHUNKS = 2        # dma_starts per x-stream per block
BUFS = {"xt": 4, "xn": 4, "y2n": 3, "y2t": 3, "sig": 8, "h": 6}
F32 = mybir.dt.float32
ALU = mybir.AluOpType
ACTF = mybir.ActivationFunctionType
NPBF = ml_dtypes.bfloat16

NCORES = 8
D = 128
TBLK = 32          # 128-node tiles per block
GBLK = 18          # graph slots per block (data max is 17)


class Cfg:
    def __init__(self, NB, TBLK=TBLK, GBLK=GBLK):
        self.NB = NB
        self.TBLK = TBLK
        self.GBLK = GBLK
        self.NTILES = NB * TBLK
        self.NNODES = self.NTILES * 128


# ---------------------------------------------------------------------------
# device program
# ---------------------------------------------------------------------------

def declare_io(nc, cfg):
    NB, GBLK = cfg.NB, cfg.GBLK
    d = {}
    d["xt"] = nc.dram_tensor("xt", [128, cfg.NTILES * 128], BF, kind="ExternalInput").ap()
    d["xn"] = nc.dram_tensor("xn", [128, cfg.NTILES * 128], BF, kind="ExternalInput").ap()
    d["sl"] = nc.dram_tensor("sl", [128, cfg.NTILES * GBLK], BF, kind="ExternalInput").ap()
    d["recip"] = nc.dram_tensor("recip", [128, NB * GBLK], F32, kind="ExternalInput").ap()
    d["fc1t"] = nc.dram_tensor("fc1t", [128, 32], BF, kind="ExternalInput").ap()
    d["fc2t"] = nc.dram_tensor("fc2t", [128, 512], BF, kind="ExternalInput").ap()
    d["wm"] = nc.dram_tensor("wm", [128, 128], F32, kind="ExternalInput").ap()
    d["b1"] = nc.dram_tensor("b1", [128, 1], F32, kind="ExternalInput").ap()
    d["ident"] = nc.dram_tensor("ident", [128, 128], BF, kind="ExternalInput").ap()
    d["outT"] = nc.dram_tensor("outT", [128, NB * GBLK], F32, kind="ExternalOutput").ap()
    return d


def build(tc, io, cfg):
    nc = tc.nc
    NB, TBLK, GBLK = cfg.NB, cfg.TBLK, cfg.GBLK
    assert TBLK % 16 == 0

    with ExitStack() as ctx:
        ep = ctx.enter_context

        consts = ep(tc.tile_pool(name="consts", bufs=1))
        # prefetch pool: block-0 x data issued before the bulky consts so the
        # PE/DVE pipeline starts as early as possible
        fc1t = consts.tile([128, 32], BF, tag="fc1t")
        nc.sync.dma_start(fc1t[:], io["fc1t"])
        fc2t = consts.tile([128, 512], BF, tag="fc2t")
        nc.sync.dma_start(fc2t[:], io["fc2t"])
        wm = consts.tile([128, 128], F32, tag="wm")
        nc.sync.dma_start(wm[:], io["wm"])
        b1c = consts.tile([128, 1], F32, tag="b1c")
        nc.sync.dma_start(b1c[:], io["b1"])
        ident = consts.tile([128, 128], BF, tag="ident")
        nc.sync.dma_start(ident[:], io["ident"])
        recip = consts.tile([128, NB * GBLK], F32, tag="recip")
        nc.sync.dma_start(recip[:], io["recip"])

        xtp = ep(tc.tile_pool(name="xt", bufs=BUFS["xt"]))
        xnp = ep(tc.tile_pool(name="xn", bufs=BUFS["xn"]))
        slp = ep(tc.tile_pool(name="sl", bufs=2))
        hp = ep(tc.tile_pool(name="h", bufs=BUFS["h"]))
        sigp = ep(tc.tile_pool(name="sig", bufs=BUFS["sig"]))
        y2np = ep(tc.tile_pool(name="y2n", bufs=BUFS["y2n"]))
        y2tp = ep(tc.tile_pool(name="y2t", bufs=BUFS["y2t"]))
        mtp = ep(tc.tile_pool(name="mt", bufs=2))
        tgp = ep(tc.tile_pool(name="tg", bufs=2))
        sdp = ep(tc.tile_pool(name="sd", bufs=2))
        cp = ep(tc.tile_pool(name="c8", bufs=2))
        outp = ep(tc.tile_pool(name="osb", bufs=2))

        # PSUM pools — 8 banks total: ph 1, pz 2 (one 2-bank tile), pyt 1,
        # pmf 2, pd 2
        php = ep(tc.tile_pool(name="ph", bufs=1, space="PSUM"))
        pzp = ep(tc.tile_pool(name="pz", bufs=(1 if PZ_WIDE else 2), space="PSUM"))
        pytp = ep(tc.tile_pool(name="pyt", bufs=1, space="PSUM"))
        pmfp = ep(tc.tile_pool(name="pmf", bufs=2, space="PSUM"))
        pdp = ep(tc.tile_pool(name="pd", bufs=2, space="PSUM"))

        for blk in range(NB):
            nbase = blk * TBLK * 128
            xt = xtp.tile([128, TBLK * 128], BF, tag="xt")
            xn = xnp.tile([128, TBLK * 128], BF, tag="xn")
            chunk = TBLK * 128 // DMA_CHUNKS
            for ci in range(DMA_CHUNKS):
                a = ci * chunk
                nc.sync.dma_start(xt[:, a:a + chunk],
                                  io["xt"][:, nbase + a:nbase + a + chunk])
                xdma = nc.scalar if XN_SCALAR_DMA else nc.sync
                xdma.dma_start(xn[:, a:a + chunk],
                               io["xn"][:, nbase + a:nbase + a + chunk])
            ssb = slp.tile([128, TBLK * GBLK], BF, tag="sl")
            nc.sync.dma_start(ssb[:], io["sl"][:, blk * TBLK * GBLK:(blk + 1) * TBLK * GBLK])

            y2n = y2np.tile([128, TBLK * 128], BF, tag="y2n")
            y2t = y2tp.tile([128, TBLK * 128], BF, tag="y2t")
            # pmf bank layout: mean [0:GBLK], fin [32:32+GBLK], tG [96:96+GBLK],
            # dots groups alternate at [128:128+8G] / [288:288+8G]
            pmf = pmfp.tile([128, 512], F32, tag="pmf")

            # ---------------- Phase A ----------------
            for g16 in range(TBLK // 16):
                ph = php.tile([128, 512], F32, tag="ph")
                # one matmul per column-group j covers 4 tiles (seg 0..3) via a
                # strided rhs AP; h lands packed as ph[32j+k, seg*128+i]
                xtg = xt[:, g16 * 2048:(g16 + 1) * 2048].rearrange(
                    "p (s j k) -> p j s k", s=4, j=4, k=128)
                for j in range(4):
                    nc.tensor.matmul(
                        ph[32 * j:32 * j + 32, 0:512],
                        fc1t[:], xtg[:, j],
                        start=True, stop=True, tile_position=(0, 32 * j))
                h16 = hp.tile([128, 512], BF, tag="h")
                relu_on_act = (RELU_ACT is True) or (RELU_ACT == "half"
                                                     and (blk * 2 + g16) % 2 == 0)
                if relu_on_act:
                    nc.scalar.activation(h16[:], ph[:], ACTF.Relu, bias=b1c[:])
                else:
                    nc.vector.tensor_scalar(h16[:], ph[:], b1c[:], 0.0,
                                            op0=ALU.add, op1=ALU.max)
                zw = 1024 if PZ_WIDE else 512          # nodes per sigmoid batch
                for t8 in range(2048 // zw):
                    # one K=128 matmul with block-diagonal fc2 computes z for
                    # 4 node-tiles at once (zeros kill cross-tile terms)
                    pz = pzp.tile([128, zw], F32, tag="pz")
                    for u in range(zw // 512):
                        t4 = t8 * (zw // 512) + u
                        nc.tensor.matmul(
                            pz[:, u * 512:(u + 1) * 512],
                            h16[:, t4 * 128:(t4 + 1) * 128], fc2t[:],
                            start=True, stop=True)
                    sig = sigp.tile([128, zw], BF, tag="sig")
                    nc.scalar.activation(sig[:], pz[:], ACTF.Sigmoid, scale=2.0)
                    c0 = (g16 * 16 + t8 * (zw // 128)) * 128
                    eng = nc.gpsimd if (g16 * 2 + t8) % 8 < Y2_GP_OF8 else nc.vector
                    eng.tensor_tensor(
                        y2n[:, c0:c0 + zw], xn[:, c0:c0 + zw], sig[:], op=ALU.mult)
                for t8 in range(2):
                    pyt = pytp.tile([128, 1024], BF, tag="pyt")
                    for k in range(8):
                        t = g16 * 16 + t8 * 8 + k
                        nc.tensor.transpose(
                            pyt[:, k * 128:(k + 1) * 128],
                            y2n[:, t * 128:(t + 1) * 128], ident[:])
                    c0 = (g16 * 16 + t8 * 8) * 128
                    # split psum->sbuf copies DVE / ACT (3:1)
                    if (g16 * 2 + t8) % 8 < CP_ACT_OF8:
                        nc.scalar.copy(y2t[:, c0:c0 + 1024], pyt[:])
                    else:
                        nc.vector.tensor_copy(y2t[:, c0:c0 + 1024], pyt[:])
                # mean accumulation for this 16-tile group (interleaved so the
                # pmf chain doesn't bunch at block end)
                for k16 in range(16):
                    t = g16 * 16 + k16
                    nc.tensor.matmul(
                        pmf[:, 0:GBLK],
                        y2n[:, t * 128:(t + 1) * 128],
                        ssb[:, t * GBLK:(t + 1) * GBLK],
                        start=(t == 0), stop=(t == TBLK - 1), skip_group_check=True)
            # ---------------- block tail A ----------------
            meant = mtp.tile([128, GBLK], F32, tag="mt")
            nc.vector.tensor_tensor(
                meant[:], pmf[:, 0:GBLK],
                recip[:, blk * GBLK:(blk + 1) * GBLK], op=ALU.mult)
            nc.tensor.matmul(pmf[:, 96:96 + GBLK], wm[:], meant[:],
                             start=True, stop=True, skip_group_check=True)
            tgt = tgp.tile([128, GBLK], BF, tag="tg")
            nc.scalar.activation(tgt[:], pmf[:, 96:96 + GBLK], ACTF.Tanh)
            # ---------------- Phase B ----------------
            DG = 512 // GBLK if GBLK > 25 else 16   # dots tiles per psum bank
            for tg in range(TBLK // DG):
                pd = pdp.tile([128, DG * GBLK], F32, tag="pd")
                for k in range(DG):
                    t = tg * DG + k
                    nc.tensor.matmul(
                        pd[:, k * GBLK:(k + 1) * GBLK],
                        y2t[:, t * 128:(t + 1) * 128], tgt[:],
                        start=True, stop=True)
                sd = sdp.tile([128, DG * GBLK], BF, tag="sd")
                nc.scalar.activation(sd[:], pd[:], ACTF.Sigmoid, scale=2.0)
                c8 = cp.tile([128, DG * GBLK], BF, tag="c8")
                nc.vector.tensor_tensor(
                    c8[:], ssb[:, tg * DG * GBLK:(tg + 1) * DG * GBLK], sd[:],
                    op=ALU.mult)
                for k in range(DG):
                    t = tg * DG + k
                    nc.tensor.matmul(
                        pmf[:, 32:32 + GBLK],
                        y2n[:, t * 128:(t + 1) * 128],
                        c8[:, k * GBLK:(k + 1) * GBLK],
                        start=(t == 0), stop=(t == TBLK - 1), skip_group_check=True)
            # ---------------- block tail B ----------------
            osb = outp.tile([128, GBLK], F32, tag="osb")
            nc.vector.tensor_scalar_mul(osb[:], pmf[:, 32:32 + GBLK], 2.0)
            nc.sync.dma_start(io["outT"][:, blk * GBLK:(blk + 1) * GBLK], osb[:])


# ---------------------------------------------------------------------------
# host-side prep / unshard
# ---------------------------------------------------------------------------

def plan_shards(batch_i32, B, ncores, tblk=TBLK, gblk=GBLK):
    cnt = np.bincount(batch_i32, minlength=B).astype(np.int64)
    starts = np.concatenate([[0], np.cumsum(cnt)])
    N = int(starts[-1])
    bounds = [0]
    for c in range(1, ncores):
        target = N * c // ncores
        g = int(np.searchsorted(starts, target))
        g = max(bounds[-1], min(g, B))
        bounds.append(g)
    bounds.append(B)
    cap = tblk * 128
    plans = []
    for c in range(ncores):
        glo, ghi = bounds[c], bounds[c + 1]
        blocks, cur, cur_nodes = [], [], 0
        for g in range(glo, ghi):
            n_g = int(cnt[g])
            assert n_g <= cap, f"graph {g} has {n_g} nodes > block capacity"
            if cur and (cur_nodes + n_g > cap or len(cur) >= gblk):
                blocks.append(cur)
                cur, cur_nodes = [], 0
            cur.append((g, int(starts[g]), n_g))
            cur_nodes += n_g
        if cur:
            blocks.append(cur)
        plans.append(blocks)
    NB = max(len(p) for p in plans)
    return plans, NB


def prep_core(x, plan, cfg):
    NB, TBLKc, GBLKc = cfg.NB, cfg.TBLK, cfg.GBLK
    xs = np.zeros((cfg.NNODES, D), np.float32)
    sl = np.zeros((cfg.NTILES * 128, GBLKc), NPBF)
    recip = np.zeros((NB, GBLKc), np.float32)
    meta = []
    for bi, blkg in enumerate(plan):
        pos = bi * TBLKc * 128
        for slot, (g, s, n_g) in enumerate(blkg):
            xs[pos:pos + n_g] = x[s:s + n_g]
            sl[pos:pos + n_g, slot] = NPBF(1.0)
            recip[bi, slot] = 2.0 / max(n_g, 1)
            meta.append((bi, slot, g))
            pos += n_g
    xs_b = xs.astype(NPBF)
    xt = np.ascontiguousarray(xs_b.T)
    xn = np.ascontiguousarray(
        xs_b.reshape(cfg.NTILES, 128, D).transpose(1, 0, 2).reshape(128, cfg.NTILES * D))
    sl_packed = np.ascontiguousarray(
        sl.reshape(cfg.NTILES, 128, GBLKc).transpose(1, 0, 2).reshape(128, cfg.NTILES * GBLKc))
    recip_b = np.ascontiguousarray(
        np.broadcast_to(recip.reshape(1, NB * GBLKc), (128, NB * GBLKc)))
    return {"xt": xt, "xn": xn, "sl": sl_packed, "recip": recip_b}, meta


def prep_consts(Wm, fc1_w, fc1_b, fc2_w, fc2_b):
    assert np.allclose(np.asarray(fc2_b, np.float32), 0.0), \
        "nonzero fc2_b not supported by this kernel build"
    fc1t = np.ascontiguousarray(np.asarray(fc1_w, np.float32).T.astype(NPBF))
    fc2t = np.zeros((128, 512), NPBF)           # block-diagonal fc2.T
    f2 = np.asarray(fc2_w, np.float32).T.astype(NPBF)
    for j in range(4):
        fc2t[32 * j:32 * j + 32, j * 128:(j + 1) * 128] = f2
    b1 = np.tile(np.asarray(fc1_b, np.float32), 4).reshape(128, 1)
    wm = np.ascontiguousarray(np.asarray(Wm, np.float32))
    ident = np.eye(128, dtype=NPBF)
    return {"fc1t": fc1t, "fc2t": fc2t, "wm": wm,
            "b1": np.ascontiguousarray(b1), "ident": ident}


def unshard(outTs, metas, B, cfg):
    out = np.zeros((B, D), np.float32)
    for outT, meta in zip(outTs, metas):
        cols = [bi * cfg.GBLK + slot for (bi, slot, g) in meta]
        gs = [g for (bi, slot, g) in meta]
        out[gs] = outT[:, cols].T
    return out


# ---------------------------------------------------------------------------
# top-level entry
# ---------------------------------------------------------------------------

_CACHE = {}


def _get_program(NB):
    key = (NB, TBLK, GBLK)
    if key not in _CACHE:
        nc = bacc.Bacc("TRN2", target_bir_lowering=False, debug=False,
                       num_devices=NCORES)
        cfg = Cfg(NB)
        io = declare_io(nc, cfg)
        with tile.TileContext(nc) as tc:
            build(tc, io, cfg)
        nc.compile()
        _CACHE[key] = (nc, cfg)
    return _CACHE[key]


def _run(inputs, trace=False):
    x = np.asarray(inputs["x"], np.float32)
    batch = np.asarray(inputs["batch"]).astype(np.int32)
    B = int(np.asarray(inputs["size"]))
    plans, NB = plan_shards(batch, B, NCORES)
    nc, cfg = _get_program(NB)
    consts = prep_consts(inputs["Wm"], inputs["fc1_w"], inputs["fc1_b"],
                         inputs["fc2_w"], inputs["fc2_b"])
    in_maps, metas = [], []
    for c in range(NCORES):
        core_in, meta = prep_core(x, plans[c], cfg)
        core_in.update(consts)
        in_maps.append(core_in)
        metas.append(meta)
    res = run_bass_kernel_spmd(nc, in_maps, core_ids=list(range(NCORES)),
                               trace=trace)
    outTs = [res.results[c]["outT"] for c in range(NCORES)]
    out = unshard(outTs, metas, B, cfg)
    return out, res


def kernel(**inputs):
    out, _ = _run(inputs, trace=False)
    return out



# revision 1
# speedup vs baseline: 1.2578x; 1.2578x over previous
"""Trainium2 Bass kernel for nn_AttentionModule (GNN message passing).

kernel(**inputs) takes the FULL unsharded inputs (as produced by
setup_inputs) and returns the FULL [B, 128] float32 output.

Strategy: data-parallel over graphs across 8 NeuronCores (batch is sorted, so
each core owns a contiguous range of graphs/nodes).  Per core, graphs are
packed into blocks of 32x128-node tiles with <= 18 graph slots; all segment
reductions are local matmuls against host-built one-hot slabs.

Math notes (big tensors bf16, accumulations fp32):
  att = tanh(z), z = relu(x@fc1.T+b1)@fc2.T (+b2==0)
  x2  = (1+att)*x = 2*sigma(2z)*x =: 2*y2       (sigmoid trick)
  meanT = (y2.T @ S) * (2/cnt);  tGT = tanh(Wm.T @ meanT)
  dots = y2 @ tGT;  coefs = sigma(2*dots);  outT = 2 * (y2.T @ (S*coefs))
"""

import sys
import numpy as np

sys.path.insert(0, "/opt/trn_rl_repo")

import ml_dtypes
from contextlib import ExitStack

import concourse.bass as bass
import concourse.bacc as bacc
import concourse.tile as tile
from concourse import mybir
from concourse.bass_utils import run_bass_kernel_spmd

BF = mybir.dt.bfloat16
# engine-balance knobs (out of 8): how many y2-multiplies go to GPSIMD,
# how many psum->sbuf copies go to ACT
Y2_GP_OF8 = 0
CP_ACT_OF8 = 0
PZ_WIDE = True        # one [128,1024] pz tile vs two-buffer [128,512]
RELU_ACT = True      # relu on ACT instead of DVE
XN_SCALAR_DMA = False # issue xn loads on the second HWDGE ring (ACT seq)
DMA_CHUNKS = 2        # dma_starts per x-stream per block
BUFS = {"xt": 4, "xn": 4, "y2n": 3, "y2t": 3, "sig": 8, "h": 6}
F32 = mybir.dt.float32
ALU = mybir.AluOpType
ACTF = mybir.ActivationFunctionType
NPBF = ml_dtypes.bfloat16

NCORES = 8
D = 128
TBLK = 32          # 128-node tiles per block
GBLK = 18          # graph slots per block (data max is 17)


class Cfg:
    def __init__(self, NB, TBLK=TBLK, GBLK=GBLK):
        self.NB = NB
        self.TBLK = TBLK
        self.GBLK = GBLK
        self.NTILES = NB * TBLK
        self.NNODES = self.NTILES * 128


# ---------------------------------------------------------------------------
# device program
# ---------------------------------------------------------------------------

def declare_io(nc, cfg):
    NB, GBLK = cfg.NB, cfg.GBLK
    d = {}
    d["xt"] = nc.dram_tensor("xt", [128, cfg.NTILES * 128], BF, kind="ExternalInput").ap()
    d["xn"] = nc.dram_tensor("xn", [128, cfg.NTILES * 128], BF, kind="ExternalInput").ap()
    d["sl"] = nc.dram_tensor("sl", [128, cfg.NTILES * GBLK], BF, kind="ExternalInput").ap()
    d["recip"] = nc.dram_tensor("recip", [128, NB * GBLK], F32, kind="ExternalInput").ap()
    d["fc1t"] = nc.dram_tensor("fc1t", [128, 32], BF, kind="ExternalInput").ap()
    d["fc2t"] = nc.dram_tensor("fc2t", [128, 512], BF, kind="ExternalInput").ap()
    d["wm"] = nc.dram_tensor("wm", [128, 128], F32, kind="ExternalInput").ap()
    d["b1"] = nc.dram_tensor("b1", [128, 1], F32, kind="ExternalInput").ap()
    d["ident"] = nc.dram_tensor("ident", [128, 128], BF, kind="ExternalInput").ap()
    d["outT"] = nc.dram_tensor("outT", [128, NB * GBLK], F32, kind="ExternalOutput").ap()
    return d


def build(tc, io, cfg):
    nc = tc.nc
    NB, TBLK, GBLK = cfg.NB, cfg.TBLK, cfg.GBLK
    assert TBLK % 16 == 0

    with ExitStack() as ctx:
        ep = ctx.enter_context

        consts = ep(tc.tile_pool(name="consts", bufs=1))
        # prefetch pool: block-0 x data issued before the bulky consts so the
        # PE/DVE pipeline starts as early as possible
        fc1t = consts.tile([128, 32], BF, tag="fc1t")
        nc.sync.dma_start(fc1t[:], io["fc1t"])
        fc2t = consts.tile([128, 512], BF, tag="fc2t")
        nc.sync.dma_start(fc2t[:], io["fc2t"])
        wm = consts.tile([128, 128], F32, tag="wm")
        nc.sync.dma_start(wm[:], io["wm"])
        b1c = consts.tile([128, 1], F32, tag="b1c")
        nc.sync.dma_start(b1c[:], io["b1"])
        ident = consts.tile([128, 128], BF, tag="ident")
        nc.sync.dma_start(ident[:], io["ident"])
        recip = consts.tile([128, NB * GBLK], F32, tag="recip")
        nc.sync.dma_start(recip[:], io["recip"])

        xtp = ep(tc.tile_pool(name="xt", bufs=BUFS["xt"]))
        xnp = ep(tc.tile_pool(name="xn", bufs=BUFS["xn"]))
        slp = ep(tc.tile_pool(name="sl", bufs=2))
        hp = ep(tc.tile_pool(name="h", bufs=BUFS["h"]))
        sigp = ep(tc.tile_pool(name="sig", bufs=BUFS["sig"]))
        y2np = ep(tc.tile_pool(name="y2n", bufs=BUFS["y2n"]))
        y2tp = ep(tc.tile_pool(name="y2t", bufs=BUFS["y2t"]))
        mtp = ep(tc.tile_pool(name="mt", bufs=2))
        tgp = ep(tc.tile_pool(name="tg", bufs=2))
        sdp = ep(tc.tile_pool(name="sd", bufs=2))
        cp = ep(tc.tile_pool(name="c8", bufs=2))
        outp = ep(tc.tile_pool(name="osb", bufs=2))

        # PSUM pools — 8 banks total: ph 1, pz 2 (one 2-bank tile), pyt 1,
        # pmf 2, pd 2
        php = ep(tc.tile_pool(name="ph", bufs=1, space="PSUM"))
        pzp = ep(tc.tile_pool(name="pz", bufs=(1 if PZ_WIDE else 2), space="PSUM"))
        pytp = ep(tc.tile_pool(name="pyt", bufs=1, space="PSUM"))
        pmfp = ep(tc.tile_pool(name="pmf", bufs=2, space="PSUM"))
        pdp = ep(tc.tile_pool(name="pd", bufs=2, space="PSUM"))

        for blk in range(NB):
            nbase = blk * TBLK * 128
            xt = xtp.tile([128, TBLK * 128], BF, tag="xt")
            xn = xnp.tile([128, TBLK * 128], BF, tag="xn")
            chunk = TBLK * 128 // DMA_CHUNKS
            for ci in range(DMA_CHUNKS):
                a = ci * chunk
                nc.sync.dma_start(xt[:, a:a + chunk],
                                  io["xt"][:, nbase + a:nbase + a + chunk])
                xdma = nc.scalar if XN_SCALAR_DMA else nc.sync
                xdma.dma_start(xn[:, a:a + chunk],
                               io["xn"][:, nbase + a:nbase + a + chunk])
            ssb = slp.tile([128, TBLK * GBLK], BF, tag="sl")
            nc.sync.dma_start(ssb[:], io["sl"][:, blk * TBLK * GBLK:(blk + 1) * TBLK * GBLK])

            y2n = y2np.tile([128, TBLK * 128], BF, tag="y2n")
            y2t = y2tp.tile([128, TBLK * 128], BF, tag="y2t")
            # pmf bank layout: mean [0:GBLK], fin [32:32+GBLK], tG [96:96+GBLK],
            # dots groups alternate at [128:128+8G] / [288:288+8G]
            pmf = pmfp.tile([128, 512], F32, tag="pmf")

            # ---------------- Phase A ----------------
            for g16 in range(TBLK // 16):
                ph = php.tile([128, 512], F32, tag="ph")
                # one matmul per column-group j covers 4 tiles (seg 0..3) via a
                # strided rhs AP; h lands packed as ph[32j+k, seg*128+i]
                xtg = xt[:, g16 * 2048:(g16 + 1) * 2048].rearrange(
                    "p (s j k) -> p j s k", s=4, j=4, k=128)
                for j in range(4):
                    nc.tensor.matmul(
                        ph[32 * j:32 * j + 32, 0:512],
                        fc1t[:], xtg[:, j],
                        start=True, stop=True, tile_position=(0, 32 * j))
                h16 = hp.tile([128, 512], BF, tag="h")
                relu_on_act = (RELU_ACT is True) or (RELU_ACT == "half"
                                                     and (blk * 2 + g16) % 2 == 0)
                if relu_on_act:
                    nc.scalar.activation(h16[:], ph[:], ACTF.Relu, bias=b1c[:])
                else:
                    nc.vector.tensor_scalar(h16[:], ph[:], b1c[:], 0.0,
                                            op0=ALU.add, op1=ALU.max)
                zw = 1024 if PZ_WIDE else 512          # nodes per sigmoid batch
                for t8 in range(2048 // zw):
                    # one K=128 matmul with block-diagonal fc2 computes z for
                    # 4 node-tiles at once (zeros kill cross-tile terms)
                    pz = pzp.tile([128, zw], F32, tag="pz")
                    for u in range(zw // 512):
                        t4 = t8 * (zw // 512) + u
                        nc.tensor.matmul(
                            pz[:, u * 512:(u + 1) * 512],
                            h16[:, t4 * 128:(t4 + 1) * 128], fc2t[:],
                            start=True, stop=True)
                    sig = sigp.tile([128, zw], BF, tag="sig")
                    nc.scalar.activation(sig[:], pz[:], ACTF.Sigmoid, scale=2.0)
                    c0 = (g16 * 16 + t8 * (zw // 128)) * 128
                    eng = nc.gpsimd if (g16 * 2 + t8) % 8 < Y2_GP_OF8 else nc.vector
                    eng.tensor_tensor(
                        y2n[:, c0:c0 + zw], xn[:, c0:c0 + zw], sig[:], op=ALU.mult)
                for t8 in range(2):
                    pyt = pytp.tile([128, 1024], BF, tag="pyt")
                    for k in range(8):
                        t = g16 * 16 + t8 * 8 + k
                        nc.tensor.transpose(
                            pyt[:, k * 128:(k + 1) * 128],
                            y2n[:, t * 128:(t + 1) * 128], ident[:])
                    c0 = (g16 * 16 + t8 * 8) * 128
                    # split psum->sbuf copies DVE / ACT (3:1)
                    if (g16 * 2 + t8) % 8 < CP_ACT_OF8:
                        nc.scalar.copy(y2t[:, c0:c0 + 1024], pyt[:])
                    else:
                        nc.vector.tensor_copy(y2t[:, c0:c0 + 1024], pyt[:])
                # mean accumulation for this 16-tile group (interleaved so the
                # pmf chain doesn't bunch at block end)
                for k16 in range(16):
                    t = g16 * 16 + k16
                    nc.tensor.matmul(
                        pmf[:, 0:GBLK],
                        y2n[:, t * 128:(t + 1) * 128],
                        ssb[:, t * GBLK:(t + 1) * GBLK],
                        start=(t == 0), stop=(t == TBLK - 1), skip_group_check=True)
            # ---------------- block tail A ----------------
            meant = mtp.tile([128, GBLK], F32, tag="mt")
            nc.vector.tensor_tensor(
                meant[:], pmf[:, 0:GBLK],
                recip[:, blk * GBLK:(blk + 1) * GBLK], op=ALU.mult)
            nc.tensor.matmul(pmf[:, 96:96 + GBLK], wm[:], meant[:],
                             start=True, stop=True, skip_group_check=True)
            tgt = tgp.tile([128, GBLK], BF, tag="tg")
            nc.scalar.activation(tgt[:], pmf[:, 96:96 + GBLK], ACTF.Tanh)
            # ---------------- Phase B ----------------
            DG = 512 // GBLK if GBLK > 25 else 16   # dots tiles per psum bank
            for tg in range(TBLK // DG):
                pd = pdp.tile([128, DG * GBLK], F32, tag="pd")
                for k in range(DG):
                    t = tg * DG + k
                    nc.tensor.matmul(
                        pd[:, k * GBLK:(k + 1) * GBLK],
                        y2t[:, t * 128:(t + 1) * 128], tgt[:],
                        start=True, stop=True)
                sd = sdp.tile([128, DG * GBLK], BF, tag="sd")
                nc.scalar.activation(sd[:], pd[:], ACTF.Sigmoid, scale=2.0)
                c8 = cp.tile([128, DG * GBLK], BF, tag="c8")
                nc.vector.tensor_tensor(
                    c8[:], ssb[:, tg * DG * GBLK:(tg + 1) * DG * GBLK], sd[:],
                    op=ALU.mult)
                for k in range(DG):
                    t = tg * DG + k
                    nc.tensor.matmul(
                        pmf[:, 32:32 + GBLK],
                        y2n[:, t * 128:(t + 1) * 128],
                        c8[:, k * GBLK:(k + 1) * GBLK],
                        start=(t == 0), stop=(t == TBLK - 1), skip_group_check=True)
            # ---------------- block tail B ----------------
            osb = outp.tile([128, GBLK], F32, tag="osb")
            nc.vector.tensor_scalar_mul(osb[:], pmf[:, 32:32 + GBLK], 2.0)
            nc.sync.dma_start(io["outT"][:, blk * GBLK:(blk + 1) * GBLK], osb[:])


# ---------------------------------------------------------------------------
# host-side prep / unshard
# ---------------------------------------------------------------------------

def plan_shards(batch_i32, B, ncores, tblk=TBLK, gblk=GBLK):
    cnt = np.bincount(batch_i32, minlength=B).astype(np.int64)
    starts = np.concatenate([[0], np.cumsum(cnt)])
    N = int(starts[-1])
    bounds = [0]
    for c in range(1, ncores):
        target = N * c // ncores
        g = int(np.searchsorted(starts, target))
        g = max(bounds[-1], min(g, B))
        bounds.append(g)
    bounds.append(B)
    cap = tblk * 128
    plans = []
    for c in range(ncores):
        glo, ghi = bounds[c], bounds[c + 1]
        blocks, cur, cur_nodes = [], [], 0
        for g in range(glo, ghi):
            n_g = int(cnt[g])
            assert n_g <= cap, f"graph {g} has {n_g} nodes > block capacity"
            if cur and (cur_nodes + n_g > cap or len(cur) >= gblk):
                blocks.append(cur)
                cur, cur_nodes = [], 0
            cur.append((g, int(starts[g]), n_g))
            cur_nodes += n_g
        if cur:
            blocks.append(cur)
        plans.append(blocks)
    NB = max(len(p) for p in plans)
    return plans, NB


def prep_core(x, plan, cfg):
    NB, TBLKc, GBLKc = cfg.NB, cfg.TBLK, cfg.GBLK
    xs = np.zeros((cfg.NNODES, D), np.float32)
    sl = np.zeros((cfg.NTILES * 128, GBLKc), NPBF)
    recip = np.zeros((NB, GBLKc), np.float32)
    meta = []
    for bi, blkg in enumerate(plan):
        pos = bi * TBLKc * 128
        for slot, (g, s, n_g) in enumerate(blkg):
            xs[pos:pos + n_g] = x[s:s + n_g]
            sl[pos:pos + n_g, slot] = NPBF(1.0)
            recip[bi, slot] = 2.0 / max(n_g, 1)
            meta.append((bi, slot, g))
            pos += n_g
    xs_b = xs.astype(NPBF)
    xt = np.ascontiguousarray(xs_b.T)
    xn = np.ascontiguousarray(
        xs_b.reshape(cfg.NTILES, 128, D).transpose(1, 0, 2).reshape(128, cfg.NTILES * D))
    sl_packed = np.ascontiguousarray(
        sl.reshape(cfg.NTILES, 128, GBLKc).transpose(1, 0, 2).reshape(128, cfg.NTILES * GBLKc))
    recip_b = np.ascontiguousarray(
        np.broadcast_to(recip.reshape(1, NB * GBLKc), (128, NB * GBLKc)))
    return {"xt": xt, "xn": xn, "sl": sl_packed, "recip": recip_b}, meta


def prep_consts(Wm, fc1_w, fc1_b, fc2_w, fc2_b):
    assert np.allclose(np.asarray(fc2_b, np.float32), 0.0), \
        "nonzero fc2_b not supported by this kernel build"
    fc1t = np.ascontiguousarray(np.asarray(fc1_w, np.float32).T.astype(NPBF))
    fc2t = np.zeros((128, 512), NPBF)           # block-diagonal fc2.T
    f2 = np.asarray(fc2_w, np.float32).T.astype(NPBF)
    for j in range(4):
        fc2t[32 * j:32 * j + 32, j * 128:(j + 1) * 128] = f2
    b1 = np.tile(np.asarray(fc1_b, np.float32), 4).reshape(128, 1)
    wm = np.ascontiguousarray(np.asarray(Wm, np.float32))
    ident = np.eye(128, dtype=NPBF)
    return {"fc1t": fc1t, "fc2t": fc2t, "wm": wm,
            "b1": np.ascontiguousarray(b1), "ident": ident}


def unshard(outTs, metas, B, cfg):
    out = np.zeros((B, D), np.float32)
    for outT, meta in zip(outTs, metas):
        cols = [bi * cfg.GBLK + slot for (bi, slot, g) in meta]
        gs = [g for (bi, slot, g) in meta]
        out[gs] = outT[:, cols].T
    return out


# ---------------------------------------------------------------------------
# top-level entry
# ---------------------------------------------------------------------------

_CACHE = {}


def _get_program(NB):
    key = (NB, TBLK, GBLK)
    if key not in _CACHE:
        nc = bacc.Bacc("TRN2", target_bir_lowering=False, debug=False,
                       num_devices=NCORES)
        cfg = Cfg(NB)
        io = declare_io(nc, cfg)
        with tile.TileContext(nc) as tc:
            build(tc, io, cfg)
        nc.compile()
        _CACHE[key] = (nc, cfg)
    return _CACHE[key]


def _run(inputs, trace=False):
    x = np.asarray(inputs["x"], np.float32)
    batch = np.asarray(inputs["batch"]).astype(np.int32)
    B = int(np.asarray(inputs["size"]))
    plans, NB = plan_shards(batch, B, NCORES)
    nc, cfg = _get_program(NB)
    consts = prep_consts(inputs["Wm"], inputs["fc1_w"], inputs["fc1_b"],
                         inputs["fc2_w"], inputs["fc2_b"])
    in_maps, metas = [], []
    for c in range(NCORES):
        core_in, meta = prep_core(x, plans[c], cfg)
        core_in.update(consts)
        in_maps.append(core_in)
        metas.append(meta)
    res = run_bass_kernel_spmd(nc, in_maps, core_ids=list(range(NCORES)),
                               trace=trace)
    outTs = [res.results[c]["outT"] for c in range(NCORES)]
    out = unshard(outTs, metas, B, cfg)
    return out, res


def kernel(**inputs):
    out, _ = _run(inputs, trace=False)
    return out

